# revision 31
# baseline (speedup 1.0000x reference)
"""Trainium2 Bass kernel: segment-mean of gathered token rows + small MLP.

Computation (matches the nn_Discriminator reference):
    hid   = transformer_hidden.reshape(-1, H)          # [V, H]
    g     = hid[indices]                               # [N_IDX, H]
    h     = segment_mean(g, segment_ids, N_SEG)        # [N_SEG, H]
    z     = gelu(h @ W_dense.T + b_dense)              # [N_SEG, H]
    out   = (z @ W_proj.T + b_proj).reshape(-1)        # [N_SEG]

Sharding: data-parallel over segments; core c owns 8 blocks of 128
segments. segment_ids is sorted, so each block's tokens are a contiguous
slice of `indices`. Per block the device:
  - dma_gathers the block's token rows (fp16, int16 indices) into SBUF
  - builds a 0/1 one-hot [token, local_seg] with a DVE compare vs iota
  - accumulates sums[seg, :] = onehot.T @ rows on the PE (PSUM f32)
  - scales by 1/count (DVE) and PE-transposes into HT [H, segs]
then dense (PE, fp16) + erf-gelu w/ bias (ACT) + projection (PE).

Blocks are processed in descending-token-count order per core so the
per-position padded chunk counts (max across cores, baked into the SPMD
program) stay tight; the host un-permutes the per-block outputs.
"""

import math
import os

import numpy as np

import concourse.bacc as bacc
import concourse.bass as bass
import concourse.mybir as mybir
import concourse.tile as tile
from concourse import bass_utils
from concourse.masks import make_identity

P = 128
B, S, H = 8, 4096, 1024
V = B * S               # 32768 gatherable rows
N_IDX = 65536
N_SEG = 8192
N_CORES = 8
SEGS_PER_CORE = N_SEG // N_CORES     # 1024
NBLK = SEGS_PER_CORE // P            # 8 seg-blocks of 128 segments per core
NJ = H // P                          # 8 partition-blocks of the hidden dim
MAXSUB = 5                           # max chunks (5*128=640 idx) per dma_gather

FP = mybir.dt.float32
I32 = mybir.dt.int32
I16 = mybir.dt.int16

# knobs test.py can poke (trace etc). Not used by the grading path.
RUN_KWARGS: dict = {}
LAST_RESULTS = None

DATA_DT = os.environ.get("KERNEL_DT", "float16")


def _subs_of(k):
    """Split k chunks into dma_gather calls of at most MAXSUB chunks."""
    out = [MAXSUB] * (k // MAXSUB)
    if k % MAXSUB:
        out.append(k % MAXSUB)
    return out


def _subs_for(b, k):
    """Call split for block position b (first block ramps up the pipeline)."""
    if b == 0 and k > 3:
        return [1, 2] + _subs_of(k - 3)
    return _subs_of(k)


def _emit(nc, cfg):
    """Emit the per-core program.

    cfg: V, K_pos (chunk count per block position), SEGS, act, dt.
    """
    cV, K_pos, SEGS = cfg["V"], cfg["K_pos"], cfg["SEGS"]
    act_fn, DT = cfg["act"], cfg["dt"]
    cNB = len(K_pos)
    KMAX = max(K_pos)
    WAVE = min(512, SEGS)
    NW = SEGS // WAVE

    hid = nc.dram_tensor("hid", [cV, H], DT, kind="ExternalInput").ap()
    # int16 gather indices in dma_gather wrapped layout: per block, per call
    # i covering chunks [c0, c0+sub), columns [c0*8, (c0+sub)*8) hold the
    # call's token t=col*16+p at partition p%16, replicated 8x down the
    # partition dim for the 8 Q7 cores.
    idxd = nc.dram_tensor("idx16", [P, cNB * KMAX * 8], I16, kind="ExternalInput").ap()
    segld = nc.dram_tensor("segl", [P, cNB * KMAX], I32, kind="ExternalInput").ap()
    invcd = nc.dram_tensor("invc", [P, cNB], FP, kind="ExternalInput").ap()
    iotad = nc.dram_tensor("iota", [P, P], I32, kind="ExternalInput").ap()
    wdtd = nc.dram_tensor("wdt", [H, H], DT, kind="ExternalInput").ap()
    bdend = nc.dram_tensor("bdense", [P, NJ], FP, kind="ExternalInput").ap()
    wptd = nc.dram_tensor("wpt", [P, NJ], DT, kind="ExternalInput").ap()
    bprojd = nc.dram_tensor("bproj", [1, 1], FP, kind="ExternalInput").ap()
    outd = nc.dram_tensor("out", [1, SEGS], FP, kind="ExternalOutput").ap()

    with tile.TileContext(nc) as tc:
        with (
            tc.tile_pool(name="const", bufs=1) as cpool,
            tc.tile_pool(name="gp", bufs=1) as gpool,
            tc.tile_pool(name="ip", bufs=1) as ipool,
            tc.tile_pool(name="wk", bufs=1) as wkpool,
            tc.tile_pool(name="pseg", bufs=1, space="PSUM") as pseg,
            tc.tile_pool(name="ptr", bufs=1, space="PSUM") as ptr,
            tc.tile_pool(name="pzt", bufs=1, space="PSUM") as pzt,
            tc.tile_pool(name="ppred", bufs=1, space="PSUM") as ppred,
        ):
            # ---- all per-block inputs in three strided DMAs on the Sync
            # HWDGE queue so the first dma_gather's indices land within ~2us
            # (HWDGE issue is ~0.6us per dma_start; don't pay it 24x) ----
            WI = KMAX * 8
            idx_all = ipool.tile([P, cNB * WI], I16)
            nc.sync.dma_start(out=idx_all[:], in_=idxd)
            segl_all = ipool.tile([P, cNB * KMAX], I32)
            nc.sync.dma_start(out=segl_all[:], in_=segld)
            invc_all = ipool.tile([P, cNB], FP)
            nc.sync.dma_start(out=invc_all[:], in_=invcd)

            # ---- small constants on the Scalar HWDGE queue ----
            iota_sb = cpool.tile([P, P], I32)
            nc.scalar.dma_start(out=iota_sb[:], in_=iotad)
            ident = cpool.tile([P, P], DT)
            make_identity(nc, ident[:])
            # weight loads are emitted lazily (after the first gathers) so the
            # gathers' DMA-completion lanes don't transitively wait on 2MB of
            # weight traffic
            wdt_sb = []
            bden_sb = wpt_sb = bproj_sb = None

            def load_weights():
                nonlocal bden_sb, wpt_sb, bproj_sb
                for k8 in range(NJ):
                    w = cpool.tile([P, H], DT, name=f"wdt_sb{k8}", tag=f"wdt{k8}")
                    nc.scalar.dma_start(out=w[:], in_=wdtd[k8 * P:(k8 + 1) * P, :])
                    wdt_sb.append(w)
                bden_sb = cpool.tile([P, NJ], FP)
                nc.scalar.dma_start(out=bden_sb[:], in_=bdend)
                wpt_sb = cpool.tile([P, NJ], DT)
                nc.scalar.dma_start(out=wpt_sb[:], in_=wptd)
                bproj_sb = cpool.tile([1, 1], FP)
                nc.scalar.dma_start(out=bproj_sb[:], in_=bprojd)
            ht_sb = []
            zt_sb = []
            for k8 in range(NJ):
                t = cpool.tile([P, SEGS], DT, name=f"ht_sb{k8}", tag=f"ht{k8}")
                ht_sb.append(t)
                t = cpool.tile([P, SEGS], DT, name=f"zt_sb{k8}", tag=f"zt{k8}")
                zt_sb.append(t)

            # ---- per block: gather + segment-sum + transpose; after the 4
            # blocks feeding a 512-seg wave, its dense+gelu+proj follows so
            # the PE never drains while later gathers stream in ----
            pred_sb = cpool.tile([1, SEGS], FP, name="pred_sb")
            blocks_per_wave = WAVE // P
            hbs = []

            def dense_wave(w):
                for j8 in range(NJ):
                    pz = pzt.tile([P, WAVE], FP, name=f"pz{w}_{j8}", tag="pzt", bufs=2)
                    for k8 in range(NJ):
                        nc.tensor.matmul(
                            out=pz[:],
                            lhsT=wdt_sb[k8][:, j8 * P:(j8 + 1) * P],
                            rhs=ht_sb[k8][:, w * WAVE:(w + 1) * WAVE],
                            start=(k8 == 0),
                            stop=(k8 == NJ - 1),
                        )
                    nc.scalar.activation(
                        out=zt_sb[j8][:, w * WAVE:(w + 1) * WAVE],
                        in_=pz[:],
                        func=act_fn,
                        bias=bden_sb[:, j8:j8 + 1],
                        scale=1.0,
                    )
                ppd = ppred.tile([1, WAVE], FP, name=f"ppd{w}", tag="ppred",
                                 bufs=1)
                for j8 in range(NJ):
                    nc.tensor.matmul(
                        out=ppd[:],
                        lhsT=wpt_sb[:, j8:j8 + 1],
                        rhs=zt_sb[j8][:, w * WAVE:(w + 1) * WAVE],
                        start=(j8 == 0),
                        stop=(j8 == NJ - 1),
                    )
                nc.vector.tensor_scalar_add(
                    out=pred_sb[0:1, w * WAVE:(w + 1) * WAVE], in0=ppd[0:1, :],
                    scalar1=bproj_sb[0:1, 0:1])

            for b in range(cNB):
                K = K_pos[b]
                subs = _subs_for(b, K)

                gts = []   # per chunk: (tile, local chunk index)
                c0 = 0
                for i, sub in enumerate(subs):
                    g = gpool.tile([P, sub * H], DT, name=f"g{b}_{i}", tag="g",
                                   bufs=8, padded_shape=[P, MAXSUB * H])
                    nc.gpsimd.dma_gather(
                        out_ap=g[:].rearrange("p (s e) -> p s e", e=H),
                        in_ap=hid,
                        idxs_ap=idx_all[:, b * WI + c0 * 8:
                                        b * WI + (c0 + sub) * 8],
                        num_idxs=sub * P,
                        num_idxs_reg=sub * P,
                        elem_size=H,
                    )
                    for cl in range(sub):
                        gts.append((g, cl))
                    c0 += sub

                ps = pseg.tile([P, H], FP, name=f"ps{b}", tag="pseg", bufs=2)
                for c in range(K):
                    g, cl = gts[c]
                    oh = wkpool.tile([P, P], DT, name=f"oh{b}_{c}", tag="oh", bufs=6)
                    nc.vector.tensor_tensor(
                        out=oh[:],
                        in0=segl_all[:, b * KMAX + c:b * KMAX + c + 1]
                            .to_broadcast((P, P)),
                        in1=iota_sb[:],
                        op=mybir.AluOpType.is_equal,
                    )
                    for hf in range(2):
                        nc.tensor.matmul(
                            out=ps[:, hf * 512:(hf + 1) * 512],
                            lhsT=oh[:],
                            rhs=g[:, cl * H + hf * 512: cl * H + (hf + 1) * 512],
                            start=(c == 0),
                            stop=(c == K - 1),
                        )

                hb = wkpool.tile([P, H], DT, name=f"hb{b}", tag="hb", bufs=2)
                nc.vector.tensor_scalar_mul(out=hb[:], in0=ps[:],
                                            scalar1=invc_all[:, b:b + 1])

                if b == 0:
                    load_weights()

                for k8 in range(NJ):
                    pt = ptr.tile([P, P], DT, name=f"pt{b}_{k8}", tag="ptr", bufs=1)
                    nc.tensor.transpose(
                        out=pt[:], in_=hb[:, k8 * P:(k8 + 1) * P], identity=ident[:]
                    )
                    nc.vector.tensor_copy(
                        out=ht_sb[k8][:, b * P:(b + 1) * P], in_=pt[:]
                    )

                if (b + 1) % blocks_per_wave == 0:
                    dense_wave((b + 1) // blocks_per_wave - 1)

            nc.sync.dma_start(out=outd, in_=pred_sb[:])
    return nc


_CACHE: dict = {}


def build(cfg_key):
    """cfg_key: (V, K_pos tuple, SEGS, act_name, dt_name). Returns compiled nc."""
    if cfg_key in _CACHE:
        return _CACHE[cfg_key]
    cV, K_pos, SEGS, act_name, dt_name = cfg_key
    cfg = {
        "V": cV, "K_pos": list(K_pos), "SEGS": SEGS,
        "act": getattr(mybir.ActivationFunctionType, act_name),
        "dt": getattr(mybir.dt, dt_name),
    }
    nc = bacc.Bacc("TRN2", target_bir_lowering=False, debug=False,
                   enable_asserts=False)
    _emit(nc, cfg)
    nc.compile()
    _CACHE[cfg_key] = nc
    return nc


def plan_blocks(seg_all, n_seg):
    """Assign 128-segment blocks to cores with an LPT-style snake over the
    descending-count order: position p of core c gets the (p*n_cores + c)-th
    (or boustrophedon-reversed) largest block. Cores stay balanced and the
    per-position chunk count (max across cores, baked into the SPMD program)
    stays tight. Returns (bounds, order[n_cores, nblk], K_pos[nblk])."""
    n_blocks = n_seg // P
    n_cores = N_CORES if n_seg == N_SEG else 1
    nblk = n_blocks // n_cores
    bounds = np.searchsorted(seg_all, np.arange(0, n_seg + P, P))
    cnts = np.diff(bounds)
    srt = np.argsort(-cnts, kind="stable")
    order = np.empty((n_cores, nblk), np.int64)
    for p in range(nblk):
        grp = srt[p * n_cores:(p + 1) * n_cores]
        order[:, p] = grp if p % 2 == 0 else grp[::-1]
    pos_max = cnts[order].max(axis=0)            # [nblk]
    K_pos = np.maximum(1, np.ceil(pos_max / P).astype(int))
    return bounds, order, K_pos


def prep_core_inputs(idx_all, seg_all, bounds, order, K_pos):
    """Host-side shard prep. Returns per-core input dict list."""
    nblk = order.shape[1]
    KMAX = int(max(K_pos))
    counts = np.bincount(seg_all, minlength=int(order.max() + 1) * P
                         ).astype(np.float64)
    invc_all = (1.0 / np.maximum(counts, 1.0)).astype(np.float32)
    per_core = []
    for c in range(order.shape[0]):
        idx16_host = np.zeros((P, nblk * KMAX * 8), np.int16)
        segl_host = np.full((P, nblk * KMAX), -1, np.int32)
        invc_host = np.ones((P, nblk), np.float32)
        WB = KMAX * 8
        for b in range(nblk):
            gb = int(order[c, b])
            k = int(K_pos[b])
            t0, t1 = bounds[gb], bounds[gb + 1]
            rows = idx_all[t0:t1]
            locs = (seg_all[t0:t1] - gb * P).astype(np.int32)
            o = np.argsort(rows, kind="stable")
            rows, locs = rows[o], locs[o]
            n = rows.shape[0]
            assert n <= k * P, f"block {gb} has {n} tokens > capacity {k * P}"
            rpad = np.zeros(k * P, np.int32)
            lpad = np.full(k * P, -1, np.int32)
            rpad[:n], lpad[:n] = rows, locs
            # matmul chunk j reads token t = j*P + p at gather slot (p, j)
            segl_host[:, b * KMAX:b * KMAX + k] = lpad.reshape(k, P).T
            c0 = 0
            for sub in _subs_for(b, k):
                tl = rpad[c0 * P:(c0 + sub) * P]
                wrapped = tl.reshape(sub * 8, 16).T.astype(np.int16)
                idx16_host[:, b * WB + c0 * 8:b * WB + (c0 + sub) * 8] = \
                    np.tile(wrapped, (8, 1))
                c0 += sub
            invc_host[:, b] = invc_all[gb * P:(gb + 1) * P]
        per_core.append({"idx16": idx16_host, "segl": segl_host,
                         "invc": invc_host})
    return per_core


def kernel(transformer_hidden, indices, segment_ids, W_dense, b_dense,
           W_proj, b_proj):
    global LAST_RESULTS
    np_dt = np.float16 if DATA_DT == "float16" else np.float32
    hid = np.ascontiguousarray(
        np.asarray(transformer_hidden, np.float32).reshape(V, H).astype(np_dt))
    idx_all = np.asarray(indices, np.int32).reshape(-1)
    seg_all = np.asarray(segment_ids, np.int32).reshape(-1)
    wdt = np.ascontiguousarray(np.asarray(W_dense, np.float32).T.astype(np_dt))
    bden = np.ascontiguousarray(
        np.asarray(b_dense, np.float32).reshape(NJ, P).T)
    wpt = np.ascontiguousarray(
        np.asarray(W_proj, np.float32).reshape(NJ, P).T.astype(np_dt))
    bproj = np.asarray(b_proj, np.float32).reshape(1, 1)
    iota = np.ascontiguousarray(
        np.broadcast_to(np.arange(P, dtype=np.int32), (P, P)))

    bounds, order, K_pos = plan_blocks(seg_all, N_SEG)
    per_core = prep_core_inputs(idx_all, seg_all, bounds, order, K_pos)
    shared = {"hid": hid, "iota": iota, "wdt": wdt, "bdense": bden,
              "wpt": wpt, "bproj": bproj}
    in_maps = [dict(shared, **pc) for pc in per_core]

    nc = build((V, tuple(int(x) for x in K_pos), SEGS_PER_CORE, "Gelu", DATA_DT))
    res = bass_utils.run_bass_kernel_spmd(
        nc, in_maps, core_ids=list(range(N_CORES)), **RUN_KWARGS)
    LAST_RESULTS = res

    out = np.empty(N_SEG, np.float32)
    for c in range(N_CORES):
        oc = np.asarray(res.results[c]["out"]).reshape(NBLK, P)
        for b in range(NBLK):
            gb = int(order[c, b])
            out[gb * P:(gb + 1) * P] = oc[b]
    return out


# revision 32
# speedup vs baseline: 1.0581x; 1.0581x over previous
"""Trainium2 Bass kernel: segment-mean of gathered token rows + small MLP.

Computation (matches the nn_Discriminator reference):
    hid   = transformer_hidden.reshape(-1, H)          # [V, H]
    g     = hid[indices]                               # [N_IDX, H]
    h     = segment_mean(g, segment_ids, N_SEG)        # [N_SEG, H]
    z     = gelu(h @ W_dense.T + b_dense)              # [N_SEG, H]
    out   = (z @ W_proj.T + b_proj).reshape(-1)        # [N_SEG]

Sharding: data-parallel over segments; core c owns 8 blocks of 128
segments. segment_ids is sorted, so each block's tokens are a contiguous
slice of `indices`. Per block the device:
  - dma_gathers the block's token rows (fp16, int16 indices) into SBUF
  - builds a 0/1 one-hot [token, local_seg] with a DVE compare vs iota
  - accumulates sums[seg, :] = onehot.T @ rows on the PE (PSUM f32)
  - scales by 1/count (DVE) and PE-transposes into HT [H, segs]
then dense (PE, fp16) + erf-gelu w/ bias (ACT) + projection (PE).

Blocks are processed in descending-token-count order per core so the
per-position padded chunk counts (max across cores, baked into the SPMD
program) stay tight; the host un-permutes the per-block outputs.
"""

import math
import os

import numpy as np

import concourse.bacc as bacc
import concourse.bass as bass
import concourse.mybir as mybir
import concourse.tile as tile
from concourse import bass_utils
from concourse.masks import make_identity

P = 128
B, S, H = 8, 4096, 1024
V = B * S               # 32768 gatherable rows
N_IDX = 65536
N_SEG = 8192
N_CORES = 8
SEGS_PER_CORE = N_SEG // N_CORES     # 1024
NBLK = SEGS_PER_CORE // P            # 8 seg-blocks of 128 segments per core
NJ = H // P                          # 8 partition-blocks of the hidden dim
MAXSUB = 5                           # max chunks (5*128=640 idx) per dma_gather

FP = mybir.dt.float32
I32 = mybir.dt.int32
I16 = mybir.dt.int16

# knobs test.py can poke (trace etc). Not used by the grading path.
RUN_KWARGS: dict = {}
LAST_RESULTS = None

DATA_DT = os.environ.get("KERNEL_DT", "float16")


def _subs_of(k):
    """Split k chunks into dma_gather calls of at most MAXSUB chunks."""
    out = [MAXSUB] * (k // MAXSUB)
    if k % MAXSUB:
        out.append(k % MAXSUB)
    return out


def _subs_for(b, k):
    """Call split for block position b (first block ramps up the pipeline)."""
    if b == 0 and k > 3:
        return [1, 2] + _subs_of(k - 3)
    return _subs_of(k)


def _emit(nc, cfg):
    """Emit the per-core program.

    cfg: V, K_pos (chunk count per block position), SEGS, act, dt.
    """
    cV, K_pos, SEGS = cfg["V"], cfg["K_pos"], cfg["SEGS"]
    act_fn, DT = cfg["act"], cfg["dt"]
    cNB = len(K_pos)
    KMAX = max(K_pos)
    WAVE = min(512, SEGS)
    NW = SEGS // WAVE

    hid = nc.dram_tensor("hid", [cV, H], DT, kind="ExternalInput").ap()
    # int16 gather indices in dma_gather wrapped layout: per block, per call
    # i covering chunks [c0, c0+sub), columns [c0*8, (c0+sub)*8) hold the
    # call's token t=col*16+p at partition p%16, replicated 8x down the
    # partition dim for the 8 Q7 cores.
    idxd = nc.dram_tensor("idx16", [P, cNB * KMAX * 8], I16, kind="ExternalInput").ap()
    segld = nc.dram_tensor("segl", [P, cNB * KMAX], I32, kind="ExternalInput").ap()
    invcd = nc.dram_tensor("invc", [P, cNB], FP, kind="ExternalInput").ap()
    iotad = nc.dram_tensor("iota", [P, P], I32, kind="ExternalInput").ap()
    wdtd = nc.dram_tensor("wdt", [H, H], DT, kind="ExternalInput").ap()
    bdend = nc.dram_tensor("bdense", [P, NJ], FP, kind="ExternalInput").ap()
    wptd = nc.dram_tensor("wpt", [P, NJ], DT, kind="ExternalInput").ap()
    bprojd = nc.dram_tensor("bproj", [1, 1], FP, kind="ExternalInput").ap()
    outd = nc.dram_tensor("out", [1, SEGS], FP, kind="ExternalOutput").ap()

    with tile.TileContext(nc) as tc:
        with (
            tc.tile_pool(name="const", bufs=1) as cpool,
            tc.tile_pool(name="gp", bufs=1) as gpool,
            tc.tile_pool(name="ip", bufs=1) as ipool,
            tc.tile_pool(name="wk", bufs=1) as wkpool,
            tc.tile_pool(name="pseg", bufs=1, space="PSUM") as pseg,
            tc.tile_pool(name="ptr", bufs=1, space="PSUM") as ptr,
            tc.tile_pool(name="pzt", bufs=1, space="PSUM") as pzt,
            tc.tile_pool(name="ppred", bufs=1, space="PSUM") as ppred,
        ):
            # ---- all per-block inputs in three strided DMAs on the Sync
            # HWDGE queue so the first dma_gather's indices land within ~2us
            # (HWDGE issue is ~0.6us per dma_start; don't pay it 24x) ----
            WI = KMAX * 8
            idx_all = ipool.tile([P, cNB * WI], I16)
            nc.sync.dma_start(out=idx_all[:], in_=idxd)
            segl_all = ipool.tile([P, cNB * KMAX], I32)
            nc.sync.dma_start(out=segl_all[:], in_=segld)
            invc_all = ipool.tile([P, cNB], FP)
            nc.sync.dma_start(out=invc_all[:], in_=invcd)

            # ---- small constants on the Scalar HWDGE queue ----
            iota_sb = cpool.tile([P, P], I32)
            nc.scalar.dma_start(out=iota_sb[:], in_=iotad)
            ident = cpool.tile([P, P], DT)
            make_identity(nc, ident[:])
            # weight loads are emitted lazily (after the first gathers) so the
            # gathers' DMA-completion lanes don't transitively wait on 2MB of
            # weight traffic
            wdt_sb = []
            bden_sb = wpt_sb = bproj_sb = None

            def load_weights():
                nonlocal bden_sb, wpt_sb, bproj_sb
                for k8 in range(NJ):
                    w = cpool.tile([P, H], DT, name=f"wdt_sb{k8}", tag=f"wdt{k8}")
                    nc.scalar.dma_start(out=w[:], in_=wdtd[k8 * P:(k8 + 1) * P, :])
                    wdt_sb.append(w)
                bden_sb = cpool.tile([P, NJ], FP)
                nc.scalar.dma_start(out=bden_sb[:], in_=bdend)
                wpt_sb = cpool.tile([P, NJ], DT)
                nc.scalar.dma_start(out=wpt_sb[:], in_=wptd)
                bproj_sb = cpool.tile([1, 1], FP)
                nc.scalar.dma_start(out=bproj_sb[:], in_=bprojd)
            ht_sb = []
            zt_sb = []
            for k8 in range(NJ):
                t = cpool.tile([P, SEGS], DT, name=f"ht_sb{k8}", tag=f"ht{k8}")
                ht_sb.append(t)
                t = cpool.tile([P, SEGS], DT, name=f"zt_sb{k8}", tag=f"zt{k8}")
                zt_sb.append(t)

            # ---- per block: gather + segment-sum + transpose; after the 4
            # blocks feeding a 512-seg wave, its dense+gelu+proj follows so
            # the PE never drains while later gathers stream in ----
            pred_sb = cpool.tile([1, SEGS], FP, name="pred_sb")
            blocks_per_wave = WAVE // P
            hbs = []

            def dense_wave(w):
                for j8 in range(NJ):
                    pz = pzt.tile([P, WAVE], FP, name=f"pz{w}_{j8}", tag="pzt", bufs=2)
                    for k8 in range(NJ):
                        nc.tensor.matmul(
                            out=pz[:],
                            lhsT=wdt_sb[k8][:, j8 * P:(j8 + 1) * P],
                            rhs=ht_sb[k8][:, w * WAVE:(w + 1) * WAVE],
                            start=(k8 == 0),
                            stop=(k8 == NJ - 1),
                        )
                    nc.scalar.activation(
                        out=zt_sb[j8][:, w * WAVE:(w + 1) * WAVE],
                        in_=pz[:],
                        func=act_fn,
                        bias=bden_sb[:, j8:j8 + 1],
                        scale=1.0,
                    )
                ppd = ppred.tile([1, WAVE], FP, name=f"ppd{w}", tag="ppred",
                                 bufs=1)
                for j8 in range(NJ):
                    nc.tensor.matmul(
                        out=ppd[:],
                        lhsT=wpt_sb[:, j8:j8 + 1],
                        rhs=zt_sb[j8][:, w * WAVE:(w + 1) * WAVE],
                        start=(j8 == 0),
                        stop=(j8 == NJ - 1),
                    )
                nc.vector.tensor_scalar_add(
                    out=pred_sb[0:1, w * WAVE:(w + 1) * WAVE], in0=ppd[0:1, :],
                    scalar1=bproj_sb[0:1, 0:1])

            for b in range(cNB):
                K = K_pos[b]
                subs = _subs_for(b, K)

                gts = []   # per chunk: (tile, local chunk index)
                c0 = 0
                for i, sub in enumerate(subs):
                    g = gpool.tile([P, sub * H], DT, name=f"g{b}_{i}", tag="g",
                                   bufs=8, padded_shape=[P, MAXSUB * H])
                    nc.gpsimd.dma_gather(
                        out_ap=g[:].rearrange("p (s e) -> p s e", e=H),
                        in_ap=hid,
                        idxs_ap=idx_all[:, b * WI + c0 * 8:
                                        b * WI + (c0 + sub) * 8],
                        num_idxs=sub * P,
                        num_idxs_reg=sub * P,
                        elem_size=H,
                    )
                    for cl in range(sub):
                        gts.append((g, cl))
                    c0 += sub

                ps = pseg.tile([P, H], FP, name=f"ps{b}", tag="pseg", bufs=1)
                for c in range(K):
                    g, cl = gts[c]
                    oh = wkpool.tile([P, P], DT, name=f"oh{b}_{c}", tag="oh", bufs=6)
                    nc.vector.tensor_tensor(
                        out=oh[:],
                        in0=segl_all[:, b * KMAX + c:b * KMAX + c + 1]
                            .to_broadcast((P, P)),
                        in1=iota_sb[:],
                        op=mybir.AluOpType.is_equal,
                    )
                    for hf in range(2):
                        nc.tensor.matmul(
                            out=ps[:, hf * 512:(hf + 1) * 512],
                            lhsT=oh[:],
                            rhs=g[:, cl * H + hf * 512: cl * H + (hf + 1) * 512],
                            start=(c == 0),
                            stop=(c == K - 1),
                        )

                hb = wkpool.tile([P, H], DT, name=f"hb{b}", tag="hb", bufs=2)
                nc.vector.tensor_scalar_mul(out=hb[:], in0=ps[:],
                                            scalar1=invc_all[:, b:b + 1])

                if b == 0:
                    load_weights()

                for k8 in range(NJ):
                    pt = ptr.tile([P, P], DT, name=f"pt{b}_{k8}", tag="ptr", bufs=3)
                    nc.tensor.transpose(
                        out=pt[:], in_=hb[:, k8 * P:(k8 + 1) * P], identity=ident[:]
                    )
                    nc.vector.tensor_copy(
                        out=ht_sb[k8][:, b * P:(b + 1) * P], in_=pt[:]
                    )

                if (b + 1) % blocks_per_wave == 0:
                    dense_wave((b + 1) // blocks_per_wave - 1)

            nc.sync.dma_start(out=outd, in_=pred_sb[:])
    return nc


_CACHE: dict = {}


def build(cfg_key):
    """cfg_key: (V, K_pos tuple, SEGS, act_name, dt_name). Returns compiled nc."""
    if cfg_key in _CACHE:
        return _CACHE[cfg_key]
    cV, K_pos, SEGS, act_name, dt_name = cfg_key
    cfg = {
        "V": cV, "K_pos": list(K_pos), "SEGS": SEGS,
        "act": getattr(mybir.ActivationFunctionType, act_name),
        "dt": getattr(mybir.dt, dt_name),
    }
    nc = bacc.Bacc("TRN2", target_bir_lowering=False, debug=False,
                   enable_asserts=False)
    _emit(nc, cfg)
    nc.compile()
    _CACHE[cfg_key] = nc
    return nc


def plan_blocks(seg_all, n_seg):
    """Assign 128-segment blocks to cores with an LPT-style snake over the
    descending-count order: position p of core c gets the (p*n_cores + c)-th
    (or boustrophedon-reversed) largest block. Cores stay balanced and the
    per-position chunk count (max across cores, baked into the SPMD program)
    stays tight. Returns (bounds, order[n_cores, nblk], K_pos[nblk])."""
    n_blocks = n_seg // P
    n_cores = N_CORES if n_seg == N_SEG else 1
    nblk = n_blocks // n_cores
    bounds = np.searchsorted(seg_all, np.arange(0, n_seg + P, P))
    cnts = np.diff(bounds)
    srt = np.argsort(-cnts, kind="stable")
    order = np.empty((n_cores, nblk), np.int64)
    for p in range(nblk):
        grp = srt[p * n_cores:(p + 1) * n_cores]
        order[:, p] = grp if p % 2 == 0 else grp[::-1]
    pos_max = cnts[order].max(axis=0)            # [nblk]
    K_pos = np.maximum(1, np.ceil(pos_max / P).astype(int))
    return bounds, order, K_pos


def prep_core_inputs(idx_all, seg_all, bounds, order, K_pos):
    """Host-side shard prep. Returns per-core input dict list."""
    nblk = order.shape[1]
    KMAX = int(max(K_pos))
    counts = np.bincount(seg_all, minlength=int(order.max() + 1) * P
                         ).astype(np.float64)
    invc_all = (1.0 / np.maximum(counts, 1.0)).astype(np.float32)
    per_core = []
    for c in range(order.shape[0]):
        idx16_host = np.zeros((P, nblk * KMAX * 8), np.int16)
        segl_host = np.full((P, nblk * KMAX), -1, np.int32)
        invc_host = np.ones((P, nblk), np.float32)
        WB = KMAX * 8
        for b in range(nblk):
            gb = int(order[c, b])
            k = int(K_pos[b])
            t0, t1 = bounds[gb], bounds[gb + 1]
            rows = idx_all[t0:t1]
            locs = (seg_all[t0:t1] - gb * P).astype(np.int32)
            o = np.argsort(rows, kind="stable")
            rows, locs = rows[o], locs[o]
            n = rows.shape[0]
            assert n <= k * P, f"block {gb} has {n} tokens > capacity {k * P}"
            rpad = np.zeros(k * P, np.int32)
            lpad = np.full(k * P, -1, np.int32)
            rpad[:n], lpad[:n] = rows, locs
            # matmul chunk j reads token t = j*P + p at gather slot (p, j)
            segl_host[:, b * KMAX:b * KMAX + k] = lpad.reshape(k, P).T
            c0 = 0
            for sub in _subs_for(b, k):
                tl = rpad[c0 * P:(c0 + sub) * P]
                wrapped = tl.reshape(sub * 8, 16).T.astype(np.int16)
                idx16_host[:, b * WB + c0 * 8:b * WB + (c0 + sub) * 8] = \
                    np.tile(wrapped, (8, 1))
                c0 += sub
            invc_host[:, b] = invc_all[gb * P:(gb + 1) * P]
        per_core.append({"idx16": idx16_host, "segl": segl_host,
                         "invc": invc_host})
    return per_core


def kernel(transformer_hidden, indices, segment_ids, W_dense, b_dense,
           W_proj, b_proj):
    global LAST_RESULTS
    np_dt = np.float16 if DATA_DT == "float16" else np.float32
    hid = np.ascontiguousarray(
        np.asarray(transformer_hidden, np.float32).reshape(V, H).astype(np_dt))
    idx_all = np.asarray(indices, np.int32).reshape(-1)
    seg_all = np.asarray(segment_ids, np.int32).reshape(-1)
    wdt = np.ascontiguousarray(np.asarray(W_dense, np.float32).T.astype(np_dt))
    bden = np.ascontiguousarray(
        np.asarray(b_dense, np.float32).reshape(NJ, P).T)
    wpt = np.ascontiguousarray(
        np.asarray(W_proj, np.float32).reshape(NJ, P).T.astype(np_dt))
    bproj = np.asarray(b_proj, np.float32).reshape(1, 1)
    iota = np.ascontiguousarray(
        np.broadcast_to(np.arange(P, dtype=np.int32), (P, P)))

    bounds, order, K_pos = plan_blocks(seg_all, N_SEG)
    per_core = prep_core_inputs(idx_all, seg_all, bounds, order, K_pos)
    shared = {"hid": hid, "iota": iota, "wdt": wdt, "bdense": bden,
              "wpt": wpt, "bproj": bproj}
    in_maps = [dict(shared, **pc) for pc in per_core]

    nc = build((V, tuple(int(x) for x in K_pos), SEGS_PER_CORE, "Gelu", DATA_DT))
    res = bass_utils.run_bass_kernel_spmd(
        nc, in_maps, core_ids=list(range(N_CORES)), **RUN_KWARGS)
    LAST_RESULTS = res

    out = np.empty(N_SEG, np.float32)
    for c in range(N_CORES):
        oc = np.asarray(res.results[c]["out"]).reshape(NBLK, P)
        for b in range(NBLK):
            gb = int(order[c, b])
            out[gb * P:(gb + 1) * P] = oc[b]
    return out


# revision 33
# speedup vs baseline: 1.0800x; 1.0207x over previous
"""Trainium2 Bass kernel: segment-mean of gathered token rows + small MLP.

Computation (matches the nn_Discriminator reference):
    hid   = transformer_hidden.reshape(-1, H)          # [V, H]
    g     = hid[indices]                               # [N_IDX, H]
    h     = segment_mean(g, segment_ids, N_SEG)        # [N_SEG, H]
    z     = gelu(h @ W_dense.T + b_dense)              # [N_SEG, H]
    out   = (z @ W_proj.T + b_proj).reshape(-1)        # [N_SEG]

Sharding: data-parallel over segments; core c owns 8 blocks of 128
segments. segment_ids is sorted, so each block's tokens are a contiguous
slice of `indices`. Per block the device:
  - dma_gathers the block's token rows (fp16, int16 indices) into SBUF
  - builds a 0/1 one-hot [token, local_seg] with a DVE compare vs iota
  - accumulates sums[seg, :] = onehot.T @ rows on the PE (PSUM f32)
  - scales by 1/count (DVE) and PE-transposes into HT [H, segs]
then dense (PE, fp16) + erf-gelu w/ bias (ACT) + projection (PE).

Blocks are processed in descending-token-count order per core so the
per-position padded chunk counts (max across cores, baked into the SPMD
program) stay tight; the host un-permutes the per-block outputs.
"""

import math
import os

import numpy as np

import concourse.bacc as bacc
import concourse.bass as bass
import concourse.mybir as mybir
import concourse.tile as tile
from concourse import bass_utils
from concourse.masks import make_identity

P = 128
B, S, H = 8, 4096, 1024
V = B * S               # 32768 gatherable rows
N_IDX = 65536
N_SEG = 8192
N_CORES = 8
SEGS_PER_CORE = N_SEG // N_CORES     # 1024
NBLK = SEGS_PER_CORE // P            # 8 seg-blocks of 128 segments per core
NJ = H // P                          # 8 partition-blocks of the hidden dim
MAXSUB = 5                           # max chunks (5*128=640 idx) per dma_gather

FP = mybir.dt.float32
I32 = mybir.dt.int32
I16 = mybir.dt.int16

# knobs test.py can poke (trace etc). Not used by the grading path.
RUN_KWARGS: dict = {}
LAST_RESULTS = None

DATA_DT = os.environ.get("KERNEL_DT", "float16")


def _subs_of(k):
    """Split k chunks into dma_gather calls of at most MAXSUB chunks."""
    out = [MAXSUB] * (k // MAXSUB)
    if k % MAXSUB:
        out.append(k % MAXSUB)
    return out


def _subs_for(b, k):
    """Call split for block position b (first block ramps up the pipeline)."""
    if b == 0 and k > 3:
        return [1, 2] + _subs_of(k - 3)
    return _subs_of(k)


def _emit(nc, cfg):
    """Emit the per-core program.

    cfg: V, K_pos (chunk count per block position), SEGS, act, dt.
    """
    cV, K_pos, SEGS = cfg["V"], cfg["K_pos"], cfg["SEGS"]
    act_fn, DT = cfg["act"], cfg["dt"]
    cNB = len(K_pos)
    KMAX = max(K_pos)
    WAVE = min(512, SEGS)
    NW = SEGS // WAVE

    hid = nc.dram_tensor("hid", [cV, H], DT, kind="ExternalInput").ap()
    # int16 gather indices in dma_gather wrapped layout: per block, per call
    # i covering chunks [c0, c0+sub), columns [c0*8, (c0+sub)*8) hold the
    # call's token t=col*16+p at partition p%16, replicated 8x down the
    # partition dim for the 8 Q7 cores.
    idxd = nc.dram_tensor("idx16", [P, cNB * KMAX * 8], I16, kind="ExternalInput").ap()
    segld = nc.dram_tensor("segl", [P, cNB * KMAX], I32, kind="ExternalInput").ap()
    invcd = nc.dram_tensor("invc", [P, cNB], FP, kind="ExternalInput").ap()
    iotad = nc.dram_tensor("iota", [P, P], I32, kind="ExternalInput").ap()
    wdtd = nc.dram_tensor("wdt", [H, H], DT, kind="ExternalInput").ap()
    bdend = nc.dram_tensor("bdense", [P, NJ], FP, kind="ExternalInput").ap()
    wptd = nc.dram_tensor("wpt", [P, NJ], DT, kind="ExternalInput").ap()
    bprojd = nc.dram_tensor("bproj", [1, 1], FP, kind="ExternalInput").ap()
    outd = nc.dram_tensor("out", [1, SEGS], FP, kind="ExternalOutput").ap()

    with tile.TileContext(nc) as tc:
        with (
            tc.tile_pool(name="const", bufs=1) as cpool,
            tc.tile_pool(name="gp", bufs=1) as gpool,
            tc.tile_pool(name="ip", bufs=1) as ipool,
            tc.tile_pool(name="wk", bufs=1) as wkpool,
            tc.tile_pool(name="pseg", bufs=1, space="PSUM") as pseg,
            tc.tile_pool(name="ptr", bufs=1, space="PSUM") as ptr,
            tc.tile_pool(name="pzt", bufs=1, space="PSUM") as pzt,
            tc.tile_pool(name="ppred", bufs=1, space="PSUM") as ppred,
        ):
            # ---- all per-block inputs in three strided DMAs on the Sync
            # HWDGE queue so the first dma_gather's indices land within ~2us
            # (HWDGE issue is ~0.6us per dma_start; don't pay it 24x) ----
            WI = KMAX * 8
            idx_all = ipool.tile([P, cNB * WI], I16)
            nc.sync.dma_start(out=idx_all[:], in_=idxd)
            segl_all = ipool.tile([P, cNB * KMAX], I32)
            nc.sync.dma_start(out=segl_all[:], in_=segld)
            invc_all = ipool.tile([P, cNB], FP)
            nc.sync.dma_start(out=invc_all[:], in_=invcd)

            # ---- small constants on the Scalar HWDGE queue ----
            iota_sb = cpool.tile([P, P], I32)
            nc.scalar.dma_start(out=iota_sb[:], in_=iotad)
            ident = cpool.tile([P, P], DT)
            make_identity(nc, ident[:])
            # weight loads are emitted lazily (after the first gathers) so the
            # gathers' DMA-completion lanes don't transitively wait on 2MB of
            # weight traffic
            wdt_sb = []
            bden_sb = wpt_sb = bproj_sb = None

            def load_weights():
                nonlocal bden_sb, wpt_sb, bproj_sb
                for k8 in range(NJ):
                    w = cpool.tile([P, H], DT, name=f"wdt_sb{k8}", tag=f"wdt{k8}")
                    nc.scalar.dma_start(out=w[:], in_=wdtd[k8 * P:(k8 + 1) * P, :])
                    wdt_sb.append(w)
                bden_sb = cpool.tile([P, NJ], FP)
                nc.scalar.dma_start(out=bden_sb[:], in_=bdend)
                wpt_sb = cpool.tile([P, NJ], DT)
                nc.scalar.dma_start(out=wpt_sb[:], in_=wptd)
                bproj_sb = cpool.tile([1, 1], FP)
                nc.scalar.dma_start(out=bproj_sb[:], in_=bprojd)
            ht_sb = []
            zt_sb = []
            for k8 in range(NJ):
                t = cpool.tile([P, SEGS], DT, name=f"ht_sb{k8}", tag=f"ht{k8}")
                ht_sb.append(t)
                t = cpool.tile([P, SEGS], DT, name=f"zt_sb{k8}", tag=f"zt{k8}")
                zt_sb.append(t)

            # ---- per block: gather + segment-sum + transpose; after the 4
            # blocks feeding a 512-seg wave, its dense+gelu+proj follows so
            # the PE never drains while later gathers stream in ----
            pred_sb = cpool.tile([1, SEGS], FP, name="pred_sb")
            blocks_per_wave = WAVE // P
            hbs = []

            def dense_wave(w):
                for j8 in range(NJ):
                    pz = pzt.tile([P, WAVE], FP, name=f"pz{w}_{j8}", tag="pzt", bufs=2)
                    for k8 in range(NJ):
                        nc.tensor.matmul(
                            out=pz[:],
                            lhsT=wdt_sb[k8][:, j8 * P:(j8 + 1) * P],
                            rhs=ht_sb[k8][:, w * WAVE:(w + 1) * WAVE],
                            start=(k8 == 0),
                            stop=(k8 == NJ - 1),
                        )
                    nc.scalar.activation(
                        out=zt_sb[j8][:, w * WAVE:(w + 1) * WAVE],
                        in_=pz[:],
                        func=act_fn,
                        bias=bden_sb[:, j8:j8 + 1],
                        scale=1.0,
                    )
                ppd = ppred.tile([1, WAVE], FP, name=f"ppd{w}", tag="ppred",
                                 bufs=1)
                for j8 in range(NJ):
                    nc.tensor.matmul(
                        out=ppd[:],
                        lhsT=wpt_sb[:, j8:j8 + 1],
                        rhs=zt_sb[j8][:, w * WAVE:(w + 1) * WAVE],
                        start=(j8 == 0),
                        stop=(j8 == NJ - 1),
                    )
                nc.vector.tensor_scalar_add(
                    out=pred_sb[0:1, w * WAVE:(w + 1) * WAVE], in0=ppd[0:1, :],
                    scalar1=bproj_sb[0:1, 0:1])

            for b in range(cNB):
                K = K_pos[b]
                subs = _subs_for(b, K)

                gts = []   # per chunk: (tile, local chunk index)
                c0 = 0
                for i, sub in enumerate(subs):
                    g = gpool.tile([P, sub * H], DT, name=f"g{b}_{i}", tag="g",
                                   bufs=8, padded_shape=[P, MAXSUB * H])
                    nc.gpsimd.dma_gather(
                        out_ap=g[:].rearrange("p (s e) -> p s e", e=H),
                        in_ap=hid,
                        idxs_ap=idx_all[:, b * WI + c0 * 8:
                                        b * WI + (c0 + sub) * 8],
                        num_idxs=sub * P,
                        num_idxs_reg=sub * P,
                        elem_size=H,
                    )
                    for cl in range(sub):
                        gts.append((g, cl))
                    c0 += sub

                ps = pseg.tile([P, H], FP, name=f"ps{b}", tag="pseg", bufs=1)
                for c in range(K):
                    g, cl = gts[c]
                    oh = wkpool.tile([P, P], DT, name=f"oh{b}_{c}", tag="oh", bufs=6)
                    nc.vector.tensor_tensor(
                        out=oh[:],
                        in0=segl_all[:, b * KMAX + c:b * KMAX + c + 1]
                            .to_broadcast((P, P)),
                        in1=iota_sb[:],
                        op=mybir.AluOpType.is_equal,
                    )
                    for hf in range(2):
                        nc.tensor.matmul(
                            out=ps[:, hf * 512:(hf + 1) * 512],
                            lhsT=oh[:],
                            rhs=g[:, cl * H + hf * 512: cl * H + (hf + 1) * 512],
                            start=(c == 0),
                            stop=(c == K - 1),
                        )

                # scale in two halves so the first transposes start after
                # half the PSUM is drained, not all of it
                hbh = []
                for h2 in range(2):
                    hb = wkpool.tile([P, H // 2], DT, name=f"hb{b}_{h2}",
                                     tag="hb", bufs=4)
                    nc.vector.tensor_scalar_mul(
                        out=hb[:], in0=ps[:, h2 * 512:(h2 + 1) * 512],
                        scalar1=invc_all[:, b:b + 1])
                    hbh.append(hb)

                if b == 0:
                    load_weights()

                for k8 in range(NJ):
                    hb = hbh[k8 // 4]
                    pt = ptr.tile([P, P], DT, name=f"pt{b}_{k8}", tag="ptr", bufs=3)
                    nc.tensor.transpose(
                        out=pt[:], in_=hb[:, (k8 % 4) * P:(k8 % 4 + 1) * P],
                        identity=ident[:]
                    )
                    nc.vector.tensor_copy(
                        out=ht_sb[k8][:, b * P:(b + 1) * P], in_=pt[:]
                    )

                if (b + 1) % blocks_per_wave == 0:
                    dense_wave((b + 1) // blocks_per_wave - 1)

            nc.sync.dma_start(out=outd, in_=pred_sb[:])
    return nc


_CACHE: dict = {}


def build(cfg_key):
    """cfg_key: (V, K_pos tuple, SEGS, act_name, dt_name). Returns compiled nc."""
    if cfg_key in _CACHE:
        return _CACHE[cfg_key]
    cV, K_pos, SEGS, act_name, dt_name = cfg_key
    cfg = {
        "V": cV, "K_pos": list(K_pos), "SEGS": SEGS,
        "act": getattr(mybir.ActivationFunctionType, act_name),
        "dt": getattr(mybir.dt, dt_name),
    }
    nc = bacc.Bacc("TRN2", target_bir_lowering=False, debug=False,
                   enable_asserts=False)
    _emit(nc, cfg)
    nc.compile()
    _CACHE[cfg_key] = nc
    return nc


def plan_blocks(seg_all, n_seg):
    """Assign 128-segment blocks to cores with an LPT-style snake over the
    descending-count order: position p of core c gets the (p*n_cores + c)-th
    (or boustrophedon-reversed) largest block. Cores stay balanced and the
    per-position chunk count (max across cores, baked into the SPMD program)
    stays tight. Returns (bounds, order[n_cores, nblk], K_pos[nblk])."""
    n_blocks = n_seg // P
    n_cores = N_CORES if n_seg == N_SEG else 1
    nblk = n_blocks // n_cores
    bounds = np.searchsorted(seg_all, np.arange(0, n_seg + P, P))
    cnts = np.diff(bounds)
    srt = np.argsort(-cnts, kind="stable")
    order = np.empty((n_cores, nblk), np.int64)
    for p in range(nblk):
        grp = srt[p * n_cores:(p + 1) * n_cores]
        order[:, p] = grp if p % 2 == 0 else grp[::-1]
    pos_max = cnts[order].max(axis=0)            # [nblk]
    K_pos = np.maximum(1, np.ceil(pos_max / P).astype(int))
    return bounds, order, K_pos


def prep_core_inputs(idx_all, seg_all, bounds, order, K_pos):
    """Host-side shard prep. Returns per-core input dict list."""
    nblk = order.shape[1]
    KMAX = int(max(K_pos))
    counts = np.bincount(seg_all, minlength=int(order.max() + 1) * P
                         ).astype(np.float64)
    invc_all = (1.0 / np.maximum(counts, 1.0)).astype(np.float32)
    per_core = []
    for c in range(order.shape[0]):
        idx16_host = np.zeros((P, nblk * KMAX * 8), np.int16)
        segl_host = np.full((P, nblk * KMAX), -1, np.int32)
        invc_host = np.ones((P, nblk), np.float32)
        WB = KMAX * 8
        for b in range(nblk):
            gb = int(order[c, b])
            k = int(K_pos[b])
            t0, t1 = bounds[gb], bounds[gb + 1]
            rows = idx_all[t0:t1]
            locs = (seg_all[t0:t1] - gb * P).astype(np.int32)
            o = np.argsort(rows, kind="stable")
            rows, locs = rows[o], locs[o]
            n = rows.shape[0]
            assert n <= k * P, f"block {gb} has {n} tokens > capacity {k * P}"
            rpad = np.zeros(k * P, np.int32)
            lpad = np.full(k * P, -1, np.int32)
            rpad[:n], lpad[:n] = rows, locs
            # matmul chunk j reads token t = j*P + p at gather slot (p, j)
            segl_host[:, b * KMAX:b * KMAX + k] = lpad.reshape(k, P).T
            c0 = 0
            for sub in _subs_for(b, k):
                tl = rpad[c0 * P:(c0 + sub) * P]
                wrapped = tl.reshape(sub * 8, 16).T.astype(np.int16)
                idx16_host[:, b * WB + c0 * 8:b * WB + (c0 + sub) * 8] = \
                    np.tile(wrapped, (8, 1))
                c0 += sub
            invc_host[:, b] = invc_all[gb * P:(gb + 1) * P]
        per_core.append({"idx16": idx16_host, "segl": segl_host,
                         "invc": invc_host})
    return per_core


def kernel(transformer_hidden, indices, segment_ids, W_dense, b_dense,
           W_proj, b_proj):
    global LAST_RESULTS
    np_dt = np.float16 if DATA_DT == "float16" else np.float32
    hid = np.ascontiguousarray(
        np.asarray(transformer_hidden, np.float32).reshape(V, H).astype(np_dt))
    idx_all = np.asarray(indices, np.int32).reshape(-1)
    seg_all = np.asarray(segment_ids, np.int32).reshape(-1)
    wdt = np.ascontiguousarray(np.asarray(W_dense, np.float32).T.astype(np_dt))
    bden = np.ascontiguousarray(
        np.asarray(b_dense, np.float32).reshape(NJ, P).T)
    wpt = np.ascontiguousarray(
        np.asarray(W_proj, np.float32).reshape(NJ, P).T.astype(np_dt))
    bproj = np.asarray(b_proj, np.float32).reshape(1, 1)
    iota = np.ascontiguousarray(
        np.broadcast_to(np.arange(P, dtype=np.int32), (P, P)))

    bounds, order, K_pos = plan_blocks(seg_all, N_SEG)
    per_core = prep_core_inputs(idx_all, seg_all, bounds, order, K_pos)
    shared = {"hid": hid, "iota": iota, "wdt": wdt, "bdense": bden,
              "wpt": wpt, "bproj": bproj}
    in_maps = [dict(shared, **pc) for pc in per_core]

    nc = build((V, tuple(int(x) for x in K_pos), SEGS_PER_CORE, "Gelu", DATA_DT))
    res = bass_utils.run_bass_kernel_spmd(
        nc, in_maps, core_ids=list(range(N_CORES)), **RUN_KWARGS)
    LAST_RESULTS = res

    out = np.empty(N_SEG, np.float32)
    for c in range(N_CORES):
        oc = np.asarray(res.results[c]["out"]).reshape(NBLK, P)
        for b in range(NBLK):
            gb = int(order[c, b])
            out[gb * P:(gb + 1) * P] = oc[b]
    return out
